# revision 24
# baseline (speedup 1.0000x reference)
"""GCN (3-layer) + GMT-style attention pooling head, distributed over 8 TRN2 NeuronCores.

Sharding: nodes + incident (dst) edges sharded across cores, graph-aligned so
both the scatter-aggregation and the attention head are core-local. Weights
replicated. Per GCN layer: each core computes t' = dis*(x@W) for its node
slice, AllGather -> full t table in shared HBM, indirect-DMA gather of
per-edge rows (within-graph degree-sorted nodes, interleaved slot layout),
segment-sum via one strided tensor_reduce per 128-node tile. The GCN norm
dis[src]*dis[dst] is separable: fold dis into t (pre) and the epilogue (post).
Attention runs fully on-chip in feature-transposed layout.
"""

import sys

sys.path.insert(0, "/opt/trn_rl_repo")

import math
from dataclasses import dataclass, field

import numpy as np

import concourse.bacc as bacc
import concourse.bass as bass
import concourse.mybir as mybir
import concourse.tile as tile
from concourse import bass_utils
from concourse.bass import ds, ts
from concourse.masks import make_identity

F32 = mybir.dt.float32
I32 = mybir.dt.int32
AF = mybir.ActivationFunctionType
ALU = mybir.AluOpType
AX = mybir.AxisListType

P = 128


@dataclass
class Cfg:
    n_cores: int = 8
    n_per: int = 400
    graphs_per_core: tuple = (32, 32, 32, 32, 32, 32, 29, 29)
    f_in: int = 3
    h1: int = 32
    c: int = 96
    heads: int = 4
    k_seeds: int = 10
    d_common: list = field(default_factory=list)

    @property
    def gmax(self):
        return max(self.graphs_per_core)

    @property
    def nloc(self):
        return self.gmax * self.n_per

    @property
    def ntiles(self):
        assert self.nloc % P == 0
        return self.nloc // P

    @property
    def ng(self):
        return self.n_cores * self.nloc

    @property
    def zrow(self):
        return self.ng

    @property
    def ttab_rows(self):
        return self.ng + P

    @property
    def dh(self):
        return self.c // self.heads

    @property
    def jchunks(self):
        out, r = [], self.n_per
        while r > 0:
            out.append(min(P, r))
            r -= out[-1]
        return out


# ----------------------------------------------------------------------------
# Host-side integer prep
# ----------------------------------------------------------------------------

def host_prep(cfg: Cfg, src, dst):
    n_cores, n_per = cfg.n_cores, cfg.n_per
    gpc = cfg.graphs_per_core
    n_real = n_per * sum(gpc)

    deg = np.bincount(dst, minlength=n_real).astype(np.int64) + 1

    core_node_start = np.concatenate([[0], np.cumsum([g * n_per for g in gpc])])

    perms = []
    pos_of = np.full(n_real, -1, dtype=np.int64)
    for cidx in range(n_cores):
        lo, hi = core_node_start[cidx], core_node_start[cidx + 1]
        local_perm = np.empty(hi - lo, dtype=np.int64)
        for g in range(gpc[cidx]):
            a = lo + g * n_per
            ids = np.arange(a, a + n_per)
            order = np.argsort(deg[ids], kind="stable")
            local_perm[g * n_per:(g + 1) * n_per] = ids[order]
        perms.append(local_perm)
        pos_of[local_perm] = cidx * cfg.nloc + np.arange(hi - lo)

    ntiles = cfg.ntiles
    d_tile = np.zeros((n_cores, ntiles), dtype=np.int64)
    for cidx in range(n_cores):
        dd = deg[perms[cidx]]
        dd = np.pad(dd, (0, cfg.nloc - len(dd)))
        d_tile[cidx] = dd.reshape(ntiles, P).max(axis=1)
    d_common = np.maximum(d_tile.max(axis=0), 1)
    cfg.d_common = [int(x) for x in d_common]
    s_cols = int(d_common.sum())

    edge_core = np.searchsorted(core_node_start[1:], dst, side="right")
    goffs, degcols, esrcs = [], [], []
    for cidx in range(n_cores):
        mask = edge_core == cidx
        e_src, e_dst = src[mask], dst[mask]
        lpos = pos_of[e_dst] - cidx * cfg.nloc
        order = np.argsort(lpos, kind="stable")
        e_src, lpos = e_src[order], lpos[order]
        src_pos = pos_of[e_src].astype(np.int32)
        counts = np.bincount(lpos, minlength=cfg.nloc)
        starts = np.concatenate([[0], np.cumsum(counts)])

        # goff laid [P, s_cols]: for tile t, columns [c0, c0+D_t), row p
        goff = np.full((P, s_cols), cfg.zrow, dtype=np.int32)
        nreal_loc = len(perms[cidx])
        c0 = 0
        for t in range(ntiles):
            Dt = int(d_common[t])
            for p in range(P):
                n = t * P + p
                if n < nreal_loc:
                    cnt = int(counts[n])
                    goff[p, c0:c0 + cnt] = src_pos[starts[n]:starts[n] + cnt]
                    goff[p, c0 + cnt] = pos_of[perms[cidx][n]]
            c0 += Dt
        goffs.append(np.ascontiguousarray(goff))
        esrcs.append(goff.copy())  # same [P, s_cols] layout, values = padded src pos

        dd = deg[perms[cidx]].astype(np.float32)
        dd = np.pad(dd, (0, cfg.nloc - len(dd)), constant_values=1.0)
        degcols.append(np.ascontiguousarray(dd.reshape(ntiles, P).T))

    return {
        "perms": perms,
        "deg": deg,
        "pos_of": pos_of,
        "goffs": goffs,
        "degcols": degcols,
        "esrcs": esrcs,
        "s_cols": s_cols,
        "core_node_start": core_node_start,
    }


# ----------------------------------------------------------------------------
# Bass program
# ----------------------------------------------------------------------------

def build_program(cfg: Cfg, s_cols: int, no_collective: bool = False):
    nc = bacc.Bacc("TRN2", target_bir_lowering=False, debug=False,
                   num_devices=cfg.n_cores)
    C, H1, FIN = cfg.c, cfg.h1, cfg.f_in
    HEADS, DH, KS = cfg.heads, cfg.dh, cfg.k_seeds
    NLOC, NT, NG, NPG = cfg.nloc, cfg.ntiles, cfg.ng, cfg.n_per
    GMAX = cfg.gmax
    DCOM = cfg.d_common
    JCH = cfg.jchunks
    NJ = len(JCH)
    HK = HEADS * KS

    din = {}

    def inp(name, shape, dtype=F32):
        din[name] = nc.dram_tensor(name, list(shape), dtype, kind="ExternalInput")
        return din[name]

    HALF = (NT + 1) // 2
    HP = HALF * P
    inp("xe", (FIN, s_cols * P))
    inp("dege", (P, s_cols))
    inp("goff", (P, s_cols), I32)
    inp("degc", (P, NT))
    inp("w1", (32 + FIN, H1))
    inp("w2", (64, H1))
    inp("w3", (64, H1))
    inp("cbias", (P, 3 * H1))

    def mab_names(pfx):
        return [(pfx + "WqA", (C, 96)), (pfx + "WkA", (C, 96)),
                (pfx + "WqB", (C, 32)), (pfx + "WkB", (C, 32)),
                (pfx + "Wv", (C, C)), (pfx + "Wo", (C, C)),
                (pfx + "bqA", (96, 1)), (pfx + "bkA", (96, 1)),
                (pfx + "bqB", (32, 1)), (pfx + "bkB", (32, 1)),
                (pfx + "bo", (C, 1)), (pfx + "bvr", (P, C)),
                (pfx + "fW", (C, C)), (pfx + "fb", (C, 1))]

    for pfx in ["m1", "m2", "m3"]:
        for nm, shp in mab_names(pfx):
            inp(nm, shp)
    inp("p1W", (C, C)); inp("p1b", (C, 1))
    inp("p2W", (C, C)); inp("p2b", (C, 1))
    inp("seed1T", (C, KS))
    inp("seed2T", (C, 1))
    inp("l1W", (C, 16)); inp("l1b", (16, 1))
    inp("l2W", (16, 2)); inp("l2b", (2, 1))

    out_d = nc.dram_tensor("out", [2, GMAX], F32, kind="ExternalOutput")

    tstage = nc.dram_tensor("tstage", [NLOC, H1], F32)
    ttab = nc.dram_tensor("ttab", [cfg.ttab_rows, H1], F32, kind="Internal",
                          addr_space="Shared")

    hT = nc.alloc_sbuf_tensor("hT", [C, NLOC], F32)
    dis_sb = nc.alloc_sbuf_tensor("dis_sb", [P, NT], F32)
    ident = nc.alloc_sbuf_tensor("ident", [P, P], F32)

    def packed(tensor, k, rows):
        """AP for chunk k of a 2-fold partition-packed [rows, NT*P] tensor."""
        g, kk = divmod(k, HALF)
        return tensor[ds(32 * g, rows), ts(kk, P)]

    wnames = ["w1", "w2", "w3", "cbias", "p1W", "p1b", "p2W", "p2b",
              "seed1T", "seed2T", "l1W", "l1b", "l2W", "l2b"]
    for pfx in ["m1", "m2", "m3"]:
        wnames += [nm for nm, _ in mab_names(pfx)]
    wsb = {}

    rg = [list(range(cfg.n_cores))]
    sqd = 1.0 / math.sqrt(DH)

    def qk_proj(pfx, ch, rhs_ap, outA, outB, app, wkp, scale):
        """q/k projection with head-3 split. Writes [96,n] and [32,n] outs."""
        for sfx, out_ap, pdim in (("A", outA, 96), ("B", outB, 32)):
            n = rhs_ap.shape[-1]
            ps = app.tile([pdim, n], F32, tag=f"qk{sfx}")
            nc.tensor.matmul(out=ps[:], lhsT=wsb[pfx + "W" + ch + sfx][:],
                             rhs=rhs_ap, start=True, stop=True)
            if scale is None:
                nc.vector.tensor_scalar(
                    out=out_ap, in0=ps[:],
                    scalar1=wsb[pfx + "b" + ch + sfx][:, 0:1],
                    scalar2=None, op0=ALU.add)
            else:
                nc.vector.tensor_scalar(
                    out=out_ap, in0=ps[:],
                    scalar1=wsb[pfx + "b" + ch + sfx][:, 0:1],
                    scalar2=scale, op0=ALU.add, op1=ALU.mult)

    with tile.TileContext(nc) as tc:
        with tc.tile_pool(name="boot", bufs=2) as boot:
            for nm in wnames:
                t_ = din[nm]
                w_ = nc.alloc_sbuf_tensor("sb_" + nm, list(t_.shape), t_.dtype)
                wsb[nm] = w_
                nc.sync.dma_start(out=w_[:], in_=t_[:])
            deg_t = boot.tile([P, NT], F32)
            nc.sync.dma_start(out=deg_t[:], in_=din["degc"][:])
            nc.scalar.activation(out=deg_t[:], in_=deg_t[:], func=AF.Sqrt)
            nc.vector.reciprocal(out=dis_sb[:], in_=deg_t[:])
            make_identity(nc, ident[:])
            zt = boot.tile([P, H1], F32)
            nc.vector.memset(zt[:], 0.0)
            nc.sync.dma_start(out=ttab[NG:NG + P, :], in_=zt[:])

        # ------------------------------------------------------------------
        # GCN layers (packed x buffers live only for this scope)
        # ------------------------------------------------------------------
        gcn_scope = tc.tile_pool(name="gcnbuf", bufs=1)
        gbuf = gcn_scope.__enter__()
        x1T = gbuf.tile([64, HP], F32, tag="x1Tp")
        x2T = gbuf.tile([64, HP], F32, tag="x2Tp")

        # ---- layer 0: messages from host-gathered x[src] via PE expand ----
        with tc.tile_pool(name="l0d", bufs=1) as l0d, \
             tc.tile_pool(name="l0x", bufs=1) as l0x, \
             tc.tile_pool(name="l0", bufs=3) as l0p, \
             tc.tile_pool(name="l0ps", bufs=4, space="PSUM") as l0ps:
            dise = l0d.tile([P, s_cols], F32, tag="disep")
            nc.sync.dma_start(out=dise[:], in_=din["dege"][:])
            nc.scalar.activation(out=dise[:], in_=dise[:], func=AF.Sqrt)
            nc.vector.reciprocal(out=dise[:], in_=dise[:])
            c0 = 0
            for t in range(NT):
                Dt = DCOM[t]
                xet = l0x.tile([FIN, Dt * P], F32, tag="xet")
                nc.sync.dma_start(out=xet[:],
                                  in_=din["xe"][:, ds(c0 * P, Dt * P)])
                msg = l0p.tile([P, Dt, H1], F32, tag="msg0")
                for cc in range(Dt):
                    pm = l0ps.tile([P, H1], F32, tag="pm")
                    nc.tensor.matmul(out=pm[:],
                                     lhsT=xet[:, ds(cc * P, P)],
                                     rhs=wsb["w1"][0:FIN, :],
                                     start=True, stop=True)
                    nc.vector.tensor_scalar(
                        out=msg[:, cc, :], in0=pm[:],
                        scalar1=dise[:, c0 + cc:c0 + cc + 1],
                        scalar2=None, op0=ALU.mult)
                acc = l0p.tile([P, H1], F32, tag="acc0")
                nc.vector.tensor_reduce(
                    out=acc[:], in_=msg[:].rearrange("p d f -> p f d"),
                    axis=AX.X, op=ALU.add)
                xt_ = l0p.tile([P, H1], F32, tag="xt0")
                nc.vector.tensor_scalar(
                    out=xt_[:], in0=acc[:], scalar1=dis_sb[:, t:t + 1],
                    scalar2=None, op0=ALU.mult)
                nc.vector.tensor_tensor(
                    out=xt_[:], in0=xt_[:],
                    in1=wsb["cbias"][:, 0:H1], op=ALU.add)
                nc.scalar.activation(out=xt_[:], in_=xt_[:], func=AF.Relu)
                pst = l0ps.tile([H1, P], F32, tag="pst0")
                nc.tensor.transpose(out=pst[:], in_=xt_[:], identity=ident[:])
                nc.vector.tensor_copy(out=hT[0:H1, ts(t, P)], in_=pst[:])
                nc.vector.tensor_copy(out=packed(x1T, t, H1), in_=pst[:])
                c0 += Dt

        goff_scope = tc.tile_pool(name="goffb", bufs=1)
        goffb = goff_scope.__enter__()
        goff_sb = goffb.tile([P, s_cols], I32, tag="goffp")
        nc.sync.dma_start(out=goff_sb[:], in_=din["goff"][:])
        for layer in range(1, 3):
            w_l = wsb[f"w{layer + 1}"]
            fin_l = H1
            x_src = [None, x1T, x2T][layer]

            with tc.tile_pool(name=f"tc{layer}", bufs=3) as tp, \
                 tc.tile_pool(name=f"tcp{layer}", bufs=3, space="PSUM") as pp:
                for k in range(NT):
                    ps = pp.tile([P, H1], F32, tag="tl")
                    nc.tensor.matmul(
                        out=ps[:],
                        lhsT=packed(x_src, k, fin_l),
                        rhs=w_l[ds(32 * (k // HALF), fin_l), :],
                        start=True, stop=True)
                    st = tp.tile([P, H1], F32, tag="tl_sb")
                    nc.scalar.activation(out=st[:], in_=ps[:], func=AF.Copy,
                                         scale=dis_sb[:, k:k + 1])
                    nc.sync.dma_start(out=tstage[ts(k, P), :], in_=st[:])

            if no_collective:
                # timing-only stand-in: move the same local bytes into ttab
                for r in range(cfg.n_cores):
                    nc.sync.dma_start(out=ttab[r * NLOC:(r + 1) * NLOC, :],
                                      in_=tstage[:])
            else:
                nc.gpsimd.collective_compute(
                    kind="AllGather", op=ALU.bypass, replica_groups=rg,
                    ins=[tstage[:]], outs=[ttab[0:NG, :]])

            with tc.tile_pool(name=f"g{layer}", bufs=3) as gp, \
                 tc.tile_pool(name=f"gp{layer}", bufs=4, space="PSUM") as gpp:
                c0 = 0
                for t in range(NT):
                    Dt = DCOM[t]
                    msg = gp.tile([P, Dt, H1], F32, tag="msg")
                    for cc in range(Dt):
                        nc.gpsimd.indirect_dma_start(
                            out=msg[:, cc, :], out_offset=None, in_=ttab[:],
                            in_offset=bass.IndirectOffsetOnAxis(
                                ap=goff_sb[:, c0 + cc:c0 + cc + 1], axis=0))
                    acc = gp.tile([P, H1], F32, tag="acc")
                    nc.vector.tensor_reduce(
                        out=acc[:], in_=msg[:].rearrange("p d f -> p f d"),
                        axis=AX.X, op=ALU.add)
                    xt_ = gp.tile([P, H1], F32, tag="xt")
                    nc.vector.tensor_scalar(
                        out=xt_[:], in0=acc[:], scalar1=dis_sb[:, t:t + 1],
                        scalar2=None, op0=ALU.mult)
                    nc.vector.tensor_tensor(
                        out=xt_[:], in0=xt_[:],
                        in1=wsb["cbias"][:, ts(layer, H1)], op=ALU.add)
                    nc.scalar.activation(out=xt_[:], in_=xt_[:], func=AF.Relu)
                    pst = gpp.tile([H1, P], F32, tag="pst")
                    nc.tensor.transpose(out=pst[:], in_=xt_[:],
                                        identity=ident[:])
                    nc.vector.tensor_copy(out=hT[ts(layer, H1), ts(t, P)],
                                          in_=pst[:])
                    if layer < 2:
                        xn = [x1T, x2T][layer]
                        nc.vector.tensor_copy(out=packed(xn, t, H1),
                                              in_=pst[:])
                    c0 += Dt

        goff_scope.__exit__(None, None, None)
        gcn_scope.__exit__(None, None, None)

        # ------------------------------------------------------------------
        # Attention head
        # ------------------------------------------------------------------
        with tc.tile_pool(name="attb", bufs=1) as big, \
             tc.tile_pool(name="attw", bufs=3) as wk:

            # --- PMA1: fully fused per-graph (h1, k, v, scores, softmax,
            # attn-out, Wo, residual, FFN) ---
            z1 = big.tile([C, GMAX * KS], F32, tag="z1")
            q1A = nc.alloc_sbuf_tensor("q1A", [96, KS], F32)
            q1B = nc.alloc_sbuf_tensor("q1B", [32, KS], F32)

            def softmax_g(t_ap):
                mx = wk.tile([P, 1], F32, tag="smx")
                nc.vector.tensor_reduce(out=mx[:], in_=t_ap, axis=AX.X,
                                        op=ALU.max)
                nc.vector.tensor_scalar(out=t_ap, in0=t_ap, scalar1=mx[:],
                                        scalar2=None, op0=ALU.subtract)
                nc.scalar.activation(out=t_ap, in_=t_ap, func=AF.Exp)
                sm = wk.tile([P, 1], F32, tag="ssm")
                nc.vector.tensor_reduce(out=sm[:], in_=t_ap, axis=AX.X,
                                        op=ALU.add)
                nc.vector.reciprocal(out=sm[:], in_=sm[:])
                nc.vector.tensor_scalar(out=t_ap, in0=t_ap, scalar1=sm[:],
                                        scalar2=None, op0=ALU.mult)

            def mab_tail(pfx, mhaT, res_ap, zout_ap, app, nq):
                pz = app.tile([C, nq], F32, tag="misc")
                nc.tensor.matmul(out=pz[:], lhsT=wsb[pfx + "Wo"][:],
                                 rhs=mhaT[:], start=True, stop=True)
                z0 = wk.tile([C, nq], F32, tag="z0")
                nc.vector.tensor_scalar(out=z0[:], in0=pz[:],
                                        scalar1=wsb[pfx + "bo"][:, 0:1],
                                        scalar2=None, op0=ALU.add)
                nc.vector.tensor_tensor(out=z0[:], in0=z0[:], in1=res_ap,
                                        op=ALU.add)
                pf = app.tile([C, nq], F32, tag="misc")
                nc.tensor.matmul(out=pf[:], lhsT=wsb[pfx + "fW"][:],
                                 rhs=z0[:], start=True, stop=True)
                ff = wk.tile([C, nq], F32, tag="ff")
                nc.scalar.activation(out=ff[:], in_=pf[:], func=AF.Relu,
                                     bias=wsb[pfx + "fb"][:, 0:1])
                nc.vector.tensor_tensor(out=zout_ap, in0=z0[:], in1=ff[:],
                                        op=ALU.add)

            with tc.tile_pool(name="pa_ps", bufs=1, space="PSUM") as app:
                qk_proj("m1", "q", wsb["seed1T"][:], q1A[:], q1B[:],
                        app, wk, sqd)
                for g in range(GMAX):
                    ph = app.tile([C, NPG], F32, tag="h1ps")
                    nc.tensor.matmul(out=ph[:], lhsT=wsb["p1W"][:],
                                     rhs=hT[:, ds(g * NPG, NPG)],
                                     start=True, stop=True)
                    h1g = wk.tile([C, NPG], F32, tag="h1g")
                    nc.scalar.activation(out=h1g[:], in_=ph[:], func=AF.Relu,
                                         bias=wsb["p1b"][:, 0:1])
                    kgA = wk.tile([96, NPG], F32, tag="kgA")
                    kgB = wk.tile([32, NPG], F32, tag="kgB")
                    qk_proj("m1", "k", h1g[:], kgA[:], kgB[:], app, wk, None)
                    vg = wk.tile([P, NJ, C], F32, tag="vg")
                    for j, jsz in enumerate(JCH):
                        pv = app.tile([P, C], F32, tag="vps")
                        nc.tensor.matmul(out=pv[:jsz, :],
                                         lhsT=h1g[:, ds(j * P, jsz)],
                                         rhs=wsb["m1Wv"][:],
                                         start=True, stop=True)
                        nc.vector.tensor_tensor(
                            out=vg[:jsz, j, :], in0=pv[:jsz, :],
                            in1=wsb["m1bvr"][:jsz, :], op=ALU.add)
                    pscA = app.tile([96, NPG], F32, tag="scpsA")
                    pscB = app.tile([KS, NPG], F32, tag="misc")
                    for h in range(3):
                        nc.tensor.matmul(out=pscA[ds(32 * h, KS), :],
                                         lhsT=q1A[ts(h, 32), :],
                                         rhs=kgA[ts(h, 32), :],
                                         start=True, stop=True)
                    nc.tensor.matmul(out=pscB[:], lhsT=q1B[:], rhs=kgB[:],
                                     start=True, stop=True)
                    scg = wk.tile([P, NPG], F32, tag="scg")
                    nc.vector.memset(scg[:], 0.0)
                    for h in range(3):
                        nc.vector.tensor_copy(out=scg[ds(32 * h, KS), :],
                                              in_=pscA[ds(32 * h, KS), :])
                    nc.vector.tensor_copy(out=scg[ds(96, KS), :], in_=pscB[:])
                    softmax_g(scg[:])
                    po = app.tile([P, C], F32, tag="po")
                    for j, jsz in enumerate(JCH):
                        pat = app.tile([P, P], F32, tag="pat")
                        nc.tensor.transpose(out=pat[:jsz, :],
                                            in_=scg[:, ds(j * P, jsz)],
                                            identity=ident[:])
                        at = wk.tile([P, P], F32, tag="at")
                        nc.vector.tensor_copy(out=at[:jsz, :], in_=pat[:jsz, :])
                        nc.tensor.matmul(out=po[:], lhsT=at[:jsz, :],
                                         rhs=vg[:jsz, j, :],
                                         start=(j == 0), stop=(j == NJ - 1))
                    mha = wk.tile([KS, C], F32, tag="mha")
                    for h in range(HEADS):
                        nc.vector.tensor_copy(out=mha[:, ts(h, DH)],
                                              in_=po[ds(32 * h, KS), ts(h, DH)])
                    pmt = app.tile([C, KS], F32, tag="misc")
                    nc.tensor.transpose(out=pmt[:], in_=mha[:],
                                        identity=ident[:KS, :KS])
                    mhaT = wk.tile([C, KS], F32, tag="mhaT")
                    nc.vector.tensor_copy(out=mhaT[:], in_=pmt[:])
                    mab_tail("m1", mhaT, wsb["seed1T"][:],
                             z1[:, ts(g, KS)], app, KS)

            def softmax_last(t_, pdim, last):
                mx = wk.tile([pdim, GMAX], F32, tag="mx")
                nc.vector.tensor_reduce(out=mx[:], in_=t_[:], axis=AX.X,
                                        op=ALU.max)
                nc.vector.tensor_tensor(
                    out=t_[:], in0=t_[:],
                    in1=mx[:].to_broadcast(
                        [pdim, GMAX, last]), op=ALU.subtract)
                nc.scalar.activation(out=t_[:], in_=t_[:], func=AF.Exp)
                sm = wk.tile([pdim, GMAX], F32, tag="sm")
                nc.vector.tensor_reduce(out=sm[:], in_=t_[:], axis=AX.X,
                                        op=ALU.add)
                nc.vector.reciprocal(out=sm[:], in_=sm[:])
                nc.vector.tensor_tensor(
                    out=t_[:], in0=t_[:],
                    in1=sm[:].to_broadcast(
                        [pdim, GMAX, last]), op=ALU.mult)

            # --- SAB ---
            GK = GMAX * KS
            q2A = big.tile([96, GK], F32, tag="q2A")
            q2B = big.tile([32, GK], F32, tag="q2B")
            k2A = big.tile([96, GK], F32, tag="k2A")
            k2B = big.tile([32, GK], F32, tag="k2B")
            v2 = big.tile([KS, GMAX, C], F32, tag="v2")
            sc2 = big.tile([P, GMAX, KS], F32, tag="sc2")
            nc.vector.memset(sc2[:], 0.0)
            with tc.tile_pool(name="sa_ps", bufs=1, space="PSUM") as app:
                qk_proj("m2", "q", z1[:], q2A[:], q2B[:], app, wk, sqd)
                qk_proj("m2", "k", z1[:], k2A[:], k2B[:], app, wk, None)
                for g in range(GMAX):
                    ps = app.tile([KS, C], F32, tag="vps2")
                    nc.tensor.matmul(out=ps[:], lhsT=z1[:, ts(g, KS)],
                                     rhs=wsb["m2Wv"][:], start=True, stop=True)
                    nc.vector.tensor_tensor(out=v2[:, g, :], in0=ps[:],
                                            in1=wsb["m2bvr"][:KS, :],
                                            op=ALU.add)
                    pscA = app.tile([96, KS], F32, tag="scps2A")
                    pscB = app.tile([KS, KS], F32, tag="scps2B")
                    for h in range(3):
                        nc.tensor.matmul(out=pscA[ds(32 * h, KS), :],
                                         lhsT=q2A[ts(h, 32), ts(g, KS)],
                                         rhs=k2A[ts(h, 32), ts(g, KS)],
                                         start=True, stop=True)
                    nc.tensor.matmul(out=pscB[:], lhsT=q2B[:, ts(g, KS)],
                                     rhs=k2B[:, ts(g, KS)],
                                     start=True, stop=True)
                    for h in range(3):
                        nc.vector.tensor_copy(out=sc2[ds(32 * h, KS), g, :],
                                              in_=pscA[ds(32 * h, KS), :])
                    nc.vector.tensor_copy(out=sc2[ds(96, KS), g, :],
                                          in_=pscB[:])

            softmax_last(sc2, P, KS)

            z2 = big.tile([C, GK], F32, tag="z2")
            with tc.tile_pool(name="sb_ps", bufs=2, space="PSUM") as app:
                for g in range(GMAX):
                    pat = app.tile([KS, P], F32, tag="pat2")
                    nc.tensor.transpose(out=pat[:], in_=sc2[:, g, :],
                                        identity=ident[:])
                    at = wk.tile([KS, P], F32, tag="at2")
                    nc.vector.tensor_copy(out=at[:], in_=pat[:])
                    po = app.tile([P, C], F32, tag="po")
                    nc.tensor.matmul(out=po[:], lhsT=at[:], rhs=v2[:, g, :],
                                     start=True, stop=True)
                    mha = wk.tile([KS, C], F32, tag="mha")
                    for h in range(HEADS):
                        nc.vector.tensor_copy(out=mha[:, ts(h, DH)],
                                              in_=po[ds(32 * h, KS), ts(h, DH)])
                    pmt = app.tile([C, KS], F32, tag="sps")
                    nc.tensor.transpose(out=pmt[:], in_=mha[:],
                                        identity=ident[:KS, :KS])
                    mhaT = wk.tile([C, KS], F32, tag="mhaT")
                    nc.vector.tensor_copy(out=mhaT[:], in_=pmt[:])
                    mab_tail("m2", mhaT, z1[:, ts(g, KS)],
                             z2[:, ts(g, KS)], app, KS)

            # --- PMA2 ---
            h2 = big.tile([C, GK], F32, tag="h2")
            k3A = big.tile([96, GK], F32, tag="k3A")
            k3B = big.tile([32, GK], F32, tag="k3B")
            v3 = big.tile([KS, GMAX, C], F32, tag="v3")
            sc3 = big.tile([P, GMAX, KS], F32, tag="sc3")
            nc.vector.memset(sc3[:], 0.0)
            q3A = nc.alloc_sbuf_tensor("q3A", [96, 1], F32)
            q3B = nc.alloc_sbuf_tensor("q3B", [32, 1], F32)
            with tc.tile_pool(name="p2_ps", bufs=1, space="PSUM") as app:
                ps = app.tile([C, GK], F32, tag="gk")
                nc.tensor.matmul(out=ps[:], lhsT=wsb["p2W"][:], rhs=z2[:],
                                 start=True, stop=True)
                nc.scalar.activation(out=h2[:], in_=ps[:], func=AF.Relu,
                                     bias=wsb["p2b"][:, 0:1])
                qk_proj("m3", "k", h2[:], k3A[:], k3B[:], app, wk, None)
                qk_proj("m3", "q", wsb["seed2T"][:], q3A[:], q3B[:],
                        app, wk, sqd)
                for g in range(GMAX):
                    ps = app.tile([KS, C], F32, tag="vps2")
                    nc.tensor.matmul(out=ps[:], lhsT=h2[:, ts(g, KS)],
                                     rhs=wsb["m3Wv"][:], start=True, stop=True)
                    nc.vector.tensor_tensor(out=v3[:, g, :], in0=ps[:],
                                            in1=wsb["m3bvr"][:KS, :],
                                            op=ALU.add)
                    pscA = app.tile([96, KS], F32, tag="scps3A")
                    pscB = app.tile([1, KS], F32, tag="scps3B")
                    for h in range(3):
                        nc.tensor.matmul(out=pscA[ds(32 * h, 1), :],
                                         lhsT=q3A[ts(h, 32), :],
                                         rhs=k3A[ts(h, 32), ts(g, KS)],
                                         start=True, stop=True)
                    nc.tensor.matmul(out=pscB[:], lhsT=q3B[:],
                                     rhs=k3B[:, ts(g, KS)],
                                     start=True, stop=True)
                    for h in range(3):
                        nc.vector.tensor_copy(out=sc3[ds(32 * h, 1), g, :],
                                              in_=pscA[ds(32 * h, 1), :])
                    nc.vector.tensor_copy(out=sc3[ds(96, 1), g, :],
                                          in_=pscB[:])

            softmax_last(sc3, P, KS)

            z3 = big.tile([C, GMAX], F32, tag="z3")
            with tc.tile_pool(name="p2b_ps", bufs=1, space="PSUM") as app:
                for g in range(GMAX):
                    pat = app.tile([KS, P], F32, tag="pat3")
                    nc.tensor.transpose(out=pat[:], in_=sc3[:, g, :],
                                        identity=ident[:])
                    at = wk.tile([KS, P], F32, tag="at3")
                    nc.vector.tensor_copy(out=at[:], in_=pat[:])
                    po = app.tile([P, C], F32, tag="po3")
                    nc.tensor.matmul(out=po[:], lhsT=at[:], rhs=v3[:, g, :],
                                     start=True, stop=True)
                    mha = wk.tile([1, C], F32, tag="mha1")
                    for h in range(HEADS):
                        nc.vector.tensor_copy(out=mha[:, ts(h, DH)],
                                              in_=po[ds(32 * h, 1), ts(h, DH)])
                    pmt = app.tile([C, 1], F32, tag="sps")
                    nc.tensor.transpose(out=pmt[:], in_=mha[:],
                                        identity=ident[:1, :1])
                    mhaT = wk.tile([C, 1], F32, tag="mhaT1")
                    nc.vector.tensor_copy(out=mhaT[:], in_=pmt[:])
                    mab_tail("m3", mhaT, wsb["seed2T"][:],
                             z3[:, g:g + 1], app, 1)

                # head MLP
                py = app.tile([16, GMAX], F32, tag="py")
                nc.tensor.matmul(out=py[:], lhsT=wsb["l1W"][:], rhs=z3[:],
                                 start=True, stop=True)
                y1 = wk.tile([16, GMAX], F32, tag="y1")
                nc.scalar.activation(out=y1[:], in_=py[:], func=AF.Relu,
                                     bias=wsb["l1b"][:, 0:1])
                py2 = app.tile([2, GMAX], F32, tag="py2")
                nc.tensor.matmul(out=py2[:], lhsT=wsb["l2W"][:], rhs=y1[:],
                                 start=True, stop=True)
                yo = wk.tile([2, GMAX], F32, tag="yo")
                nc.vector.tensor_scalar(out=yo[:], in0=py2[:],
                                        scalar1=wsb["l2b"][:, 0:1],
                                        scalar2=None, op0=ALU.add)
                nc.sync.dma_start(out=out_d[:], in_=yo[:])

    nc.compile()
    return nc


# ----------------------------------------------------------------------------
# in_maps assembly
# ----------------------------------------------------------------------------

def build_in_maps(cfg: Cfg, x, params, prep):
    def rep(a):
        return np.ascontiguousarray(np.asarray(a, dtype=np.float32))

    shared = {}
    def dup32(w, rows):
        w = np.asarray(w, np.float32)
        out = np.zeros((32 + rows, w.shape[1]), np.float32)
        out[:w.shape[0]] = w
        out[32:32 + w.shape[0]] = w
        return out

    shared["w1"] = dup32(params["conv1"]["W"], cfg.f_in)
    shared["w2"] = dup32(params["conv2"]["W"], 32)
    shared["w3"] = dup32(params["conv3"]["W"], 32)
    cb = np.concatenate([
        np.broadcast_to(np.asarray(params[f"conv{i+1}"]["b"], np.float32),
                        (P, cfg.h1)) for i in range(3)], axis=1)
    shared["cbias"] = rep(cb)

    dh = cfg.dh

    def pad_heads_w(w):
        w = np.asarray(w, np.float32)
        a = np.zeros((w.shape[0], 96), np.float32)
        for h in range(3):
            a[:, 32 * h:32 * h + dh] = w[:, dh * h:dh * (h + 1)]
        b = np.zeros((w.shape[0], 32), np.float32)
        b[:, :dh] = w[:, 3 * dh:4 * dh]
        return a, b

    def pad_heads_b(bb):
        bb = np.asarray(bb, np.float32).reshape(-1)
        a = np.zeros((96, 1), np.float32)
        for h in range(3):
            a[32 * h:32 * h + dh, 0] = bb[dh * h:dh * (h + 1)]
        b = np.zeros((32, 1), np.float32)
        b[:dh, 0] = bb[3 * dh:4 * dh]
        return a, b

    def mab_fill(pfx, mp):
        shared[pfx + "WqA"], shared[pfx + "WqB"] = pad_heads_w(mp["Wq"])
        shared[pfx + "WkA"], shared[pfx + "WkB"] = pad_heads_w(mp["Wk"])
        shared[pfx + "Wv"] = rep(mp["Wv"])
        shared[pfx + "Wo"] = rep(mp["Wo"])
        shared[pfx + "bqA"], shared[pfx + "bqB"] = pad_heads_b(mp["bq"])
        shared[pfx + "bkA"], shared[pfx + "bkB"] = pad_heads_b(mp["bk"])
        shared[pfx + "bo"] = rep(mp["bo"]).reshape(cfg.c, 1)
        shared[pfx + "bvr"] = rep(np.broadcast_to(
            np.asarray(mp["bv"], np.float32), (P, cfg.c)))
        shared[pfx + "fW"] = rep(mp["lin"]["W"])
        shared[pfx + "fb"] = rep(mp["lin"]["b"]).reshape(cfg.c, 1)

    mab_fill("m1", params["pma1"]["mab"])
    mab_fill("m2", params["sab"]["mab"])
    mab_fill("m3", params["pma2"]["mab"])
    shared["p1W"] = rep(params["pma1"]["lin"]["W"])
    shared["p1b"] = rep(params["pma1"]["lin"]["b"]).reshape(cfg.c, 1)
    shared["p2W"] = rep(params["pma2"]["lin"]["W"])
    shared["p2b"] = rep(params["pma2"]["lin"]["b"]).reshape(cfg.c, 1)
    shared["seed1T"] = rep(np.asarray(params["pma1"]["seed"],
                                      np.float32)[0].T)
    shared["seed2T"] = rep(np.asarray(params["pma2"]["seed"],
                                      np.float32)[0].T)
    shared["l1W"] = rep(params["lin1"]["W"])
    shared["l1b"] = rep(params["lin1"]["b"]).reshape(16, 1)
    shared["l2W"] = rep(params["lin2"]["W"])
    shared["l2b"] = rep(params["lin2"]["b"]).reshape(2, 1)

    # global padded-position lookup tables for x and deg
    ng = cfg.ng
    xglob = np.zeros((ng + 1, cfg.f_in), np.float32)
    degglob = np.ones(ng + 1, np.float32)
    x = np.asarray(x, np.float32)
    deg_all = prep["deg"]
    for c in range(cfg.n_cores):
        perm = prep["perms"][c]
        xglob[c * cfg.nloc:c * cfg.nloc + len(perm)] = x[perm]
        degglob[c * cfg.nloc:c * cfg.nloc + len(perm)] = deg_all[perm]

    in_maps = []
    for c in range(cfg.n_cores):
        m = dict(shared)
        goff = prep["goffs"][c]
        gidx = np.minimum(goff, ng)   # zrow == ng
        m["xe"] = np.ascontiguousarray(xglob[gidx.T.reshape(-1)].T)
        m["dege"] = np.ascontiguousarray(degglob[gidx])
        m["goff"] = goff
        m["degc"] = prep["degcols"][c]
        in_maps.append(m)
    return in_maps


# ----------------------------------------------------------------------------
# public entry
# ----------------------------------------------------------------------------

def run(cfg, x, edge_index, params, use_sim=False, want_results=None):
    src = np.asarray(edge_index[0], dtype=np.int64)
    dst = np.asarray(edge_index[1], dtype=np.int64)
    prep = host_prep(cfg, src, dst)
    nc = build_program(cfg, prep["s_cols"])
    in_maps = build_in_maps(cfg, x, params, prep)

    if use_sim:
        from concourse.bass_interp import MultiCoreSim
        sim = MultiCoreSim(nc, num_cores=cfg.n_cores, trace=False,
                           require_finite=False, require_nnan=False)
        for cidx, cs in enumerate(sim.cores.values()):
            for k, v in in_maps[cidx].items():
                cs.tensor(k)[:] = v
        sim.simulate()
        results = [{"out": np.array(cs.tensor("out"))}
                   for cs in sim.cores.values()]
        res_obj = None
    else:
        res_obj = bass_utils.run_bass_kernel_spmd(
            nc, in_maps, core_ids=list(range(cfg.n_cores)),
            **(want_results or {}))
        results = res_obj.results

    outs = []
    for c in range(cfg.n_cores):
        o = np.asarray(results[c]["out"])
        outs.append(o[:, :cfg.graphs_per_core[c]].T)
    y = np.concatenate(outs, axis=0).astype(np.float32)
    return (y, res_obj) if want_results is not None else y


def kernel(x, edge_index, batch, params):
    cfg = Cfg()
    return run(cfg, np.asarray(x), np.asarray(edge_index), params)
